# revision 1
# baseline (speedup 1.0000x reference)
"""Chamfer distance kernel for Trainium2 (8 NeuronCores).

Problem: pred/target [4, 8192, 3] f32 -> scalar
  mean_b( mean_m min_n ||p_bm - q_bn||^2 + mean_n min_m ||p_bm - q_bn||^2 )

Strategy (one "side" per core; 4 batches x 2 directions = 8 cores):
  Each core owns one (batch, direction) pair and computes, for each of its
  8192 "own" points, the min squared distance to all 8192 "other" points.

  Distances are produced on the TensorEngine as K=8 matmuls using the
  identity ||p-q||^2 = -2 p.q + ||p||^2 + ||q||^2:
      lhsT rows: [-2x, -2y, -2z, n_hi, n_lo, 1, 1, 0]   (own points)
      rhs  rows: [ x,   y,  z,  1,    1,  n_hi, n_lo, 0] (other points)
  Inputs are fp16; norms are split hi/lo into two fp16 values so the norm
  contribution keeps ~2^-22 precision; fp16 products are exact in the fp32
  PSUM accumulation. Because K=8 uses only 8 of the PE's 128 rows — and
  this part runs the PE cold at 1.2 GHz — four matmuls are packed into
  disjoint 32-row groups via tile_position, running concurrently (~4x).
  Host-side prep replicates lhsT/rhs at partition offsets 0/32/64/96.

  Each m-tile (128 own points) streams 4 "rounds" of 2048 distances into
  two rotating [128,2048] PSUM tiles (4 banks each). ScalarE stages 3
  rounds to fp16 SBUF; VectorE pair-mins (PSUM,staged) at 1x and
  (staged,staged) at fp16 2x. The merge/fold ladder is batched across
  m-tile pairs, and the final fold+reduce across GRP=8 m-tiles, to
  amortize per-op overheads. The 3-staged/1-direct split balances
  ScalarE vs VectorE (both ~equally busy, measured).
"""

import numpy as np

import concourse.bacc as bacc
import concourse.mybir as mybir
import concourse.tile as tile
from concourse import bass_utils

P = 128          # partitions / m-tile size
NPTS = 8192      # points per cloud
B = 4            # batch
K = 8            # matmul contraction (padded)
MT = NPTS // P   # 64 m-tiles
RND = 2048       # columns per round (one 4-bank PSUM tile, 4 packed MMs)
MM_N = 512       # matmul free dim (one PSUM bank of fp32)
GRP = 8          # m-tiles whose tails are batched into one fold+reduce

F16 = mybir.dt.float16
F32 = mybir.dt.float32
MIN = mybir.AluOpType.min


def _emit_round(nc, ps, lt4, rt4, t, r):
    """4 row-group-packed K=8 matmuls filling one [128, 2048] PSUM tile."""
    for i in range(4):
        n0 = r * RND + i * MM_N
        nc.tensor.matmul(
            ps[:, i * MM_N:(i + 1) * MM_N],
            lt4[32 * i:32 * i + K, t * P:(t + 1) * P],
            rt4[32 * i:32 * i + K, n0:n0 + MM_N],
            start=True,
            stop=True,
            tile_position=(32 * i, 0),
        )


def _emit_mtile_l1(nc, psum, stg, lt4, rt4, t, x0_slot, x1_slot):
    """One m-tile's matmuls + L1 pair-mins into the pair buffers.

    Rounds 0-2 are staged to fp16 SBUF by ScalarE; round 3 is drained by
    VectorE against the early-staged st0 (1x); st1/st2 pair at fp16 2x.
    """
    staged = []
    for r in range(3):
        ps = psum.tile([P, RND], F32, tag="ps")
        _emit_round(nc, ps, lt4, rt4, t, r)
        st = stg.tile([P, RND], F16, tag="st")
        nc.scalar.copy(st[:], ps[:])
        staged.append(st)
    ps3 = psum.tile([P, RND], F32, tag="ps")
    _emit_round(nc, ps3, lt4, rt4, t, 3)

    nc.vector.tensor_tensor(x0_slot, ps3[:], staged[0][:], op=MIN)
    nc.vector.tensor_tensor(x1_slot, staged[1][:], staged[2][:], op=MIN)


def _build_nc():
    nc = bacc.Bacc(
        "TRN2", target_bir_lowering=False, debug=False, num_devices=8
    )
    lhsT_d = nc.dram_tensor("lhsT", [P, NPTS], F16, kind="ExternalInput")
    rhs_d = nc.dram_tensor("rhs", [P, NPTS], F16, kind="ExternalInput")
    mins_d = nc.dram_tensor("mins", [P, MT], F32, kind="ExternalOutput")

    with tile.TileContext(nc) as tc:
        with (
            tc.tile_pool(name="const", bufs=1) as const,
            tc.tile_pool(name="psum", bufs=2, space="PSUM") as psum,
            tc.tile_pool(name="stg", bufs=6) as stg,
            tc.tile_pool(name="xpool", bufs=3) as xpool,
            tc.tile_pool(name="wpool", bufs=2) as wpool,
        ):
            lt4 = const.tile([P, NPTS], F16)
            rt4 = const.tile([P, NPTS], F16)
            res = const.tile([P, MT], F32)
            # first m-tile's weights + first rounds' rhs land first so the
            # PE starts streaming before the bulk of the input DMA finishes
            nc.sync.dma_start(lt4[:, :P], lhsT_d.ap()[:, :P])
            nc.sync.dma_start(rt4[:, :RND], rhs_d.ap()[:, :RND])
            nc.sync.dma_start(lt4[:, P:], lhsT_d.ap()[:, P:])
            nc.sync.dma_start(rt4[:, RND:], rhs_d.ap()[:, RND:])

            W = MM_N  # wbuf slot width (512)
            for g in range(MT // GRP):
                wbuf = wpool.tile([P, GRP, W], F16, tag="w")
                for j in range(GRP // 2):
                    # process an m-tile pair; batch its merge ladder
                    x0b = xpool.tile([P, 2, RND], F16, tag="x0")
                    x1b = xpool.tile([P, 2, RND], F16, tag="x1")
                    for h in range(2):
                        t = g * GRP + 2 * j + h
                        _emit_mtile_l1(
                            nc, psum, stg, lt4, rt4, t,
                            x0b[:, h, :], x1b[:, h, :],
                        )
                    z2 = xpool.tile([P, 2, RND], F16, tag="z2")
                    nc.vector.tensor_tensor(z2[:], x0b[:], x1b[:], op=MIN)
                    zz2 = xpool.tile([P, 2, RND // 2], F16, tag="zz2")
                    nc.vector.tensor_tensor(
                        zz2[:], z2[:, :, :RND // 2], z2[:, :, RND // 2:],
                        op=MIN,
                    )
                    nc.vector.tensor_tensor(
                        wbuf[:, 2 * j:2 * j + 2, :],
                        zz2[:, :, :RND // 4], zz2[:, :, RND // 4:], op=MIN,
                    )
                # batched tail: fold 512->256->128, reduce 128->1 per m-tile
                v = wpool.tile([P, GRP, W // 2], F16, tag="v")
                nc.vector.tensor_tensor(
                    v[:], wbuf[:, :, :W // 2], wbuf[:, :, W // 2:], op=MIN
                )
                u = wpool.tile([P, GRP, W // 4], F16, tag="u")
                nc.vector.tensor_tensor(
                    u[:], v[:, :, :W // 4], v[:, :, W // 4:], op=MIN
                )
                nc.vector.tensor_reduce(
                    res[:, g * GRP:(g + 1) * GRP], u[:],
                    axis=mybir.AxisListType.X, op=MIN,
                )

            nc.sync.dma_start(mins_d.ap(), res[:])

    nc.compile()
    return nc


_NC_CACHE = []


def _get_nc():
    if not _NC_CACHE:
        _NC_CACHE.append(_build_nc())
    return _NC_CACHE[0]


def _prep_side(own, other):
    """Build lhsT [128, N] and rhs [128, N] fp16 with the K=8 row content
    replicated at partition offsets 0/32/64/96 for row-group packing."""
    o16 = own.astype(np.float16)
    t16 = other.astype(np.float16)
    o32 = o16.astype(np.float32)
    t32 = t16.astype(np.float32)
    on = (o32 * o32).sum(-1)       # fp32 norms of the fp16-rounded points
    tn = (t32 * t32).sum(-1)
    on_hi = on.astype(np.float16)
    on_lo = (on - on_hi.astype(np.float32)).astype(np.float16)
    tn_hi = tn.astype(np.float16)
    tn_lo = (tn - tn_hi.astype(np.float32)).astype(np.float16)

    n = own.shape[0]
    lhsT = np.zeros((K, n), np.float16)
    lhsT[0:3] = (-2.0 * o32).astype(np.float16).T
    lhsT[3] = on_hi
    lhsT[4] = on_lo
    lhsT[5] = 1.0
    lhsT[6] = 1.0
    rhs = np.zeros((K, n), np.float16)
    rhs[0:3] = t16.T
    rhs[3] = 1.0
    rhs[4] = 1.0
    rhs[5] = tn_hi
    rhs[6] = tn_lo

    lhsT4 = np.zeros((P, n), np.float16)
    rhs4 = np.zeros((P, n), np.float16)
    for g in range(4):
        lhsT4[32 * g:32 * g + K] = lhsT
        rhs4[32 * g:32 * g + K] = rhs
    return lhsT4, rhs4


def _in_maps_for(pred, target):
    pred = np.asarray(pred, dtype=np.float32)
    target = np.asarray(target, dtype=np.float32)
    in_maps = []
    for b in range(B):
        for d in range(2):
            own, other = (
                (pred[b], target[b]) if d == 0 else (target[b], pred[b])
            )
            lhsT4, rhs4 = _prep_side(own, other)
            in_maps.append({"lhsT": lhsT4, "rhs": rhs4})
    return in_maps


def kernel(pred, target):
    in_maps = _in_maps_for(pred, target)
    nc = _get_nc()
    r = bass_utils.run_bass_kernel_spmd(nc, in_maps, core_ids=list(range(8)))

    total = 0.0
    for core_res in r.results:
        total += core_res["mins"].astype(np.float64).mean()
    return np.array(total / B, dtype=np.float32)



# revision 6
# speedup vs baseline: 4.8100x; 4.8100x over previous
"""Chamfer distance kernel for Trainium2 (8 NeuronCores).

Problem: pred/target [4, 8192, 3] f32 -> scalar
  mean_b( mean_m min_n ||p_bm - q_bn||^2 + mean_n min_m ||p_bm - q_bn||^2 )

Strategy (one "side" per core; 4 batches x 2 directions = 8 cores):
  Each core owns one (batch, direction) pair. Instead of scanning all
  8192 candidates per point (the brute-force baseline), both clouds are
  sorted by radius on the host and each 128-point tile only scans a
  C=1024 rank-window of candidates. Exactness is certified per point via
  the reverse triangle inequality: the true NN of p lies within radius
  |R_p - R_q| <= dist, so an upper bound u_p on the NN distance (min over
  +-128 rank neighbors, host-computed) gives a rank interval that must be
  inside the tile's window. Points whose certificate does not fit (a few
  dozen isolated outliers that dominate the metric's tail) are routed to
  2 "rescue" tiles that scan the full 8192 candidates. The device output
  for a rescued point's natural slot is ignored on the host.

  Distances are produced on the TensorEngine as K=8 matmuls using
  ||p-q||^2 = -2 p.q + ||p||^2 + ||q||^2 (fp16 inputs, norms split hi/lo,
  fp32 PSUM). Four matmuls are packed into disjoint 32-row groups via
  tile_position so they stream concurrently; a [128, 2048] PSUM buffer
  holds two natural windows (or half a rescue round). Consumption per
  window: ScalarE stages the upper 512 columns to fp16 SBUF, then a
  single fused VectorE tensor_tensor_reduce computes
  min(psum_lo, staged_hi) and min-reduces it straight into the result
  column - no min ladder, one vector op per tile. Rescue tiles chain 4
  such reduces over the full range via the scalar (init) operand.
"""

import numpy as np

import concourse.bacc as bacc
import concourse.mybir as mybir
import concourse.tile as tile
from concourse import bass_utils

P = 128          # partitions / tile size
NPTS = 8192      # points per cloud
B = 4            # batch
K = 8            # matmul contraction (padded)
MT = NPTS // P   # 64 natural tiles
C = 1024         # natural window width
NRESC = 2        # full-range rescue tiles
NT = MT + NRESC  # 66 result columns
MM_N = 512       # matmul free dim (one PSUM bank of fp32)
NB = 128         # host neighbor-bound half-width (certificate)
BIG = 60000.0    # min-reduce init (> any distance, fp16-safe)

F16 = mybir.dt.float16
F32 = mybir.dt.float32
MIN = mybir.AluOpType.min


def _win_off(t):
    """Compile-time window offset of natural tile t (rank-centered)."""
    return min(max(t * P + P // 2 - C // 2, 0), NPTS - C)


def _build_nc():
    nc = bacc.Bacc(
        "TRN2", target_bir_lowering=False, debug=False, num_devices=8
    )
    lhsT_d = nc.dram_tensor("lhsT", [P, NT * P], F16, kind="ExternalInput")
    rhs_d = nc.dram_tensor("rhs", [P, NPTS], F16, kind="ExternalInput")
    mins_d = nc.dram_tensor("mins", [P, NT], F32, kind="ExternalOutput")

    with tile.TileContext(nc) as tc:
        with (
            tc.tile_pool(name="const", bufs=1) as const,
            tc.tile_pool(name="psum", bufs=2, space="PSUM") as psum,
            tc.tile_pool(name="stg", bufs=4) as stg,
            tc.tile_pool(name="scr", bufs=2) as scr,
            tc.tile_pool(name="acc", bufs=4) as acc,
        ):
            lt4 = const.tile([P, NT * P], F16)
            rt4 = const.tile([P, NPTS], F16)
            res = const.tile([P, NT], F32)
            # first pair's weights + first windows' rhs land first so the
            # PE starts streaming before the bulk of the input DMA finishes
            nc.sync.dma_start(lt4[:, :2 * P], lhsT_d.ap()[:, :2 * P])
            nc.sync.dma_start(rt4[:, :2048], rhs_d.ap()[:, :2048])
            nc.sync.dma_start(lt4[:, 2 * P:], lhsT_d.ap()[:, 2 * P:])
            nc.sync.dma_start(rt4[:, 2048:], rhs_d.ap()[:, 2048:])

            # natural tiles: groups of 8, pairs share one PSUM buffer;
            # per tile one TT-min (PSUM half vs staged half) into a wbuf
            # slot, then a batched fold+reduce tail (baseline-proven ops)
            GRP = 8
            W = C // 2
            for gi in range(MT // GRP):
                wbuf = scr.tile([P, GRP, W], F16, tag="w")
                for j in range(GRP // 2):
                    ps = psum.tile([P, 2, C], F32, tag="ps")
                    for h in range(2):
                        t = gi * GRP + 2 * j + h
                        o = _win_off(t)
                        for i in range(C // MM_N):
                            g = 2 * h + i
                            nc.tensor.matmul(
                                ps[:, h, i * MM_N:(i + 1) * MM_N],
                                lt4[32 * g:32 * g + K, t * P:(t + 1) * P],
                                rt4[32 * g:32 * g + K, o + i * MM_N:
                                    o + (i + 1) * MM_N],
                                start=True,
                                stop=True,
                                tile_position=(32 * g, 0),
                            )
                    st = stg.tile([P, 2, W], F16, tag="st")
                    nc.scalar.copy(st[:, 0, :], ps[:, 0, W:])
                    nc.scalar.copy(st[:, 1, :], ps[:, 1, W:])
                    for h in range(2):
                        nc.vector.tensor_tensor(
                            wbuf[:, 2 * j + h, :],
                            ps[:, h, :W], st[:, h, :], op=MIN,
                        )
                v = scr.tile([P, GRP, W // 2], F16, tag="v")
                nc.vector.tensor_tensor(
                    v[:], wbuf[:, :, :W // 2], wbuf[:, :, W // 2:], op=MIN
                )
                u = scr.tile([P, GRP, W // 4], F16, tag="u")
                nc.vector.tensor_tensor(
                    u[:], v[:, :, :W // 4], v[:, :, W // 4:], op=MIN
                )
                nc.vector.tensor_reduce(
                    res[:, gi * GRP:(gi + 1) * GRP], u[:],
                    axis=mybir.AxisListType.X, op=MIN,
                )

            # rescue tiles: full-range scan = one group's worth of rounds,
            # folded to a single column
            n_rounds = NPTS // (2 * C)
            for r in range(NRESC):
                tcol = MT + r
                wbuf = scr.tile([P, GRP, W], F16, tag="w")
                for rnd in range(n_rounds):
                    ps = psum.tile([P, 2, C], F32, tag="ps")
                    for q in range(4):
                        c0 = rnd * 2 * C + q * MM_N
                        nc.tensor.matmul(
                            ps[:, q // 2, (q % 2) * MM_N:
                               (q % 2 + 1) * MM_N],
                            lt4[32 * q:32 * q + K,
                                tcol * P:(tcol + 1) * P],
                            rt4[32 * q:32 * q + K, c0:c0 + MM_N],
                            start=True,
                            stop=True,
                            tile_position=(32 * q, 0),
                        )
                    st = stg.tile([P, 2, W], F16, tag="st")
                    nc.scalar.copy(st[:, 0, :], ps[:, 0, W:])
                    nc.scalar.copy(st[:, 1, :], ps[:, 1, W:])
                    for h in range(2):
                        nc.vector.tensor_tensor(
                            wbuf[:, 2 * rnd + h, :],
                            ps[:, h, :W], st[:, h, :], op=MIN,
                        )
                v = scr.tile([P, GRP, W // 2], F16, tag="v")
                nc.vector.tensor_tensor(
                    v[:], wbuf[:, :, :W // 2], wbuf[:, :, W // 2:], op=MIN
                )
                u = scr.tile([P, GRP, W // 4], F16, tag="u")
                nc.vector.tensor_tensor(
                    u[:], v[:, :, :W // 4], v[:, :, W // 4:], op=MIN
                )
                a8 = acc.tile([P, GRP], F32, tag="a8")
                nc.vector.tensor_reduce(
                    a8[:], u[:], axis=mybir.AxisListType.X, op=MIN,
                )
                nc.vector.tensor_reduce(
                    res[:, tcol:tcol + 1], a8[:],
                    axis=mybir.AxisListType.X, op=MIN,
                )

            nc.sync.dma_start(mins_d.ap(), res[:])

    nc.compile()
    return nc


_NC_CACHE = []


def _get_nc():
    if not _NC_CACHE:
        _NC_CACHE.append(_build_nc())
    return _NC_CACHE[0]


def _features(pts16, n):
    """K=8 feature rows for own (lhsT) or other (rhs) points."""
    p32 = pts16.astype(np.float32)
    nrm = (p32 * p32).sum(-1)
    hi = nrm.astype(np.float16)
    lo = (nrm - hi.astype(np.float32)).astype(np.float16)
    own = np.zeros((K, n), np.float16)
    own[0:3] = (-2.0 * p32).astype(np.float16).T
    own[3] = hi
    own[4] = lo
    own[5] = 1.0
    own[6] = 1.0
    oth = np.zeros((K, n), np.float16)
    oth[0:3] = pts16.T
    oth[3] = 1.0
    oth[4] = 1.0
    oth[5] = hi
    oth[6] = lo
    return own, oth


def _prep_pair(own, other):
    """Sort by radius, certify windows, route failures to rescue tiles.

    Returns (lhsT4, rhs4, natural_ok mask in sorted own order, rescue_idx
    into sorted own order, overflow_idx).
    """
    o16 = own.astype(np.float16)
    t16 = other.astype(np.float16)
    o32 = o16.astype(np.float32)
    t32 = t16.astype(np.float32)
    on = (o32 * o32).sum(-1)
    tn = (t32 * t32).sum(-1)
    oi = np.argsort(on, kind="stable")
    ti = np.argsort(tn, kind="stable")
    o_s16 = o16[oi]
    t_s16 = t16[ti]
    o_s = o32[oi]
    t_s = t32[ti]
    Ro = np.sqrt(on[oi])
    Rt = np.sqrt(tn[ti])

    # neighbor upper bound on NN dist^2 (certificate)
    n = NPTS
    u = np.full(n, np.inf, np.float32)
    base = np.arange(n)
    for s in range(-NB, NB + 1):
        idx = np.clip(base + s, 0, n - 1)
        d = ((o_s - t_s[idx]) ** 2).sum(-1)
        u = np.minimum(u, d)
    su = np.sqrt(u) * 1.001 + 1e-5
    lo_rank = np.searchsorted(Rt, Ro - su, side="left")
    hi_rank = np.searchsorted(Rt, Ro + su, side="right")
    wlo = np.array([_win_off(t) for t in range(MT)])[base // P]
    ok = (lo_rank >= wlo) & (hi_rank <= wlo + C)
    fail = np.where(~ok)[0]
    rescue = fail[:NRESC * P]
    overflow = fail[NRESC * P:]

    # own-side lhsT columns: 64 natural tiles (sorted order) + rescue pts
    own_cols = np.concatenate(
        [o_s16, o_s16[rescue],
         np.broadcast_to(o_s16[:1], (NRESC * P - len(rescue), 3))]
    )
    lhsT, _ = _features(own_cols, NT * P)
    _, rhs = _features(t_s16, n)

    lhsT4 = np.zeros((P, NT * P), np.float16)
    rhs4 = np.zeros((P, n), np.float16)
    for g in range(4):
        lhsT4[32 * g:32 * g + K] = lhsT
        rhs4[32 * g:32 * g + K] = rhs
    return lhsT4, rhs4, ok, rescue, overflow, o_s, t_s, on[oi], tn[ti]


def _in_maps_for(pred, target):
    pred = np.asarray(pred, dtype=np.float32)
    target = np.asarray(target, dtype=np.float32)
    in_maps = []
    meta = []
    for b in range(B):
        for d in range(2):
            own, other = (
                (pred[b], target[b]) if d == 0 else (target[b], pred[b])
            )
            lhsT4, rhs4, ok, rescue, overflow, o_s, t_s, on_s, tn_s = (
                _prep_pair(own, other)
            )
            in_maps.append({"lhsT": lhsT4, "rhs": rhs4})
            meta.append((ok, rescue, overflow, o_s, t_s, on_s, tn_s))
    return in_maps, meta


def kernel(pred, target):
    in_maps, meta = _in_maps_for(pred, target)
    nc = _get_nc()
    r = bass_utils.run_bass_kernel_spmd(nc, in_maps, core_ids=list(range(8)))

    total = 0.0
    for core_res, (ok, rescue, overflow, o_s, t_s, on_s, tn_s) in zip(
        r.results, meta
    ):
        mins = core_res["mins"].astype(np.float64)  # [P, NT]
        nat = mins[:, :MT].T.reshape(-1)            # sorted own order
        s = nat[ok].sum()
        resc = mins[:, MT:].T.reshape(-1)           # rescue slots
        s += resc[:len(rescue)].sum()
        # overflow (certificate routing ran out of rescue capacity):
        # exact host fallback for the handful of remaining points
        for i in overflow:
            d = on_s[i] + tn_s - 2.0 * (o_s[i] @ t_s.T)
            s += float(d.min())
        total += s / NPTS
    return np.array(total / B, dtype=np.float32)
